# revision 30
# baseline (speedup 1.0000x reference)
"""Trainium2 Bass kernel for nn_Encoder_48017734369665 (PointNet-style
segment-reduce encoder).

Network (B=16 clouds, N=131072 points, ragged via npts):
    h  = relu(bn1(W1 @ x))            [128, N]
    f  = bn2(W2 @ h)                  [256, N]
    g  = segment_max(f)               [B, 256]
    h3 = relu(bn3(W3 @ [g[seg]; f]))  [512, N]
    h4 = bn4(W4 @ h3)                 [1024, N]
    out = segment_max(h4)             [B, 1024]

Strategy:
  * Inference-mode batchnorms fold into the affine layers on the host; the
    f-path of layer 3 folds further: W32 = W3f @ W2 lets layer 3 contract
    directly over h (K=128), so f itself is never materialized for layer 3.
  * All matmul operands are bf16 (full-rate 1 col/cycle streaming plus fast
    weight load, which fp32 operands do not get); PSUM accumulation stays
    fp32. Measured end-to-end error ~5e-3 vs the 2e-2 gate. fp8 DoubleRow
    was evaluated and rejected: e4m3 quantization of W4/h3 alone measures
    4e-2 — over the gate.
  * Each core owns 2 whole clouds, paired so their sizes sum to exactly
    N/8 = 16384 points (true for this npts distribution). The core runs 32
    full 512-point tiles plus one 256-wide half-tile holding both clouds'
    sub-tile remainders (each side padded with duplicates of its own
    points, which is max-invariant); slot-to-cloud assignment rides in the
    per-core mask data, so the SPMD program is identical on every core and
    no full padding tile is ever computed.
  * Pass 1 per tile: L1 -> h (bf16, stashed in SBUF for the whole core),
    L2 pair into PSUM, one DVE reduce -> per-tile maxes Mt. The combine
    masks Mt per cloud slot (SPMD-safe for any per-core cloud split),
    builds g, then c = W3g' g + b3' + W3f' b2', and broadcasts c into the
    per-tile bias table Cb via tiny K=1 matmuls.
  * Pass 2 per tile: L3 = W32 @ h (4 matmuls) -> relu(. + Cb) -> h3 (bf16),
    L4 = 32 matmuls into 2-bank PSUM pairs, one DVE reduce per pair ->
    per-tile maxes V, DMA'd out every 2 tiles. Host combines per-tile maxes
    into [B, 1024] (adding b4'): no cross-device communication at all.
  * One PSUM pool scope with two bank tags ([128,2,F]x3 shared by the L2
    pairs and L4 pairs; [128,F]x2 for L1/combine/L3) lets L3 matmuls of the
    first tiles interleave into the combine's DVE window; L3 of tile t+2 is
    emitted in halves between L4 pair streams of tile t, keeping the PE
    gap-free at the 216 ns/matmul bf16 roofline through all of pass 2.
"""

import numpy as np

EPS = 1e-5
B = 16
N = 131072
F = 512  # points per tile (fp32 moving-operand / PSUM-bank limit)
NCORES = 8
NEG = -1.0e30


def _fold_bn(W, b, g, be, m, v):
    """bn(W@x + b) == W' @ x + b' with W' = s*W, b' = s*(b-m)+be, s=g/sqrt(v+eps)."""
    s = g / np.sqrt(v + EPS)
    return (s[:, None] * W).astype(np.float32), (s * (b - m) + be).astype(np.float32)


def _cloud_ranges(npts):
    """Per-cloud [start, end) column ranges exactly as the reference's
    jnp.repeat(..., total_repeat_length=N) maps points to clouds: truncate
    if sum > N, extend the last cloud if sum < N."""
    npts = np.maximum(np.asarray(npts, np.int64), 0)
    ends = np.minimum(np.cumsum(npts), N)
    starts = np.concatenate([[0], ends[:-1]])
    ends = ends.copy()
    ends[-1] = N  # pad semantics: trailing points belong to the last cloud
    return [(int(s), int(e)) for s, e in zip(starts, ends)]


HALF = 256  # width of the shared remainder tile (tile T-1) in exact mode
_EXACT = [False]


def _plan(npts):
    """Pair clouds 2-per-core. In exact mode (each pair sums to N/NCORES
    points, true for this problem's npts), a core runs 32 full tiles plus
    one HALF-wide tile that holds both clouds' sub-tile remainders (each
    padded with duplicates of its own points, which is max-invariant).

    Returns (T, slots): slots[c] = [(cid, col_segments, tile_list)] where
    col_segments rebuild the core's xs and tile_list are the Mt/V slots
    owned by that cloud.
    """
    ranges = _cloud_ranges(npts)
    sizes = [e - s for s, e in ranges]
    order = np.argsort(sizes)[::-1]
    pairs = [(int(order[i]), int(order[2 * NCORES - 1 - i])) for i in range(NCORES)]
    per_core = N // NCORES
    _EXACT[0] = all(sizes[a] + sizes[b] == per_core for a, b in pairs)
    if not _EXACT[0]:
        # fallback: per-cloud whole tiles (original scheme)
        ktiles = [max(1, -(-sizes[b] // F)) for b in range(2 * NCORES)]
        T = max(ktiles[a] + ktiles[b] for a, b in pairs)
        slots = []
        for a, b in pairs:
            ka = ktiles[a]
            kb = T - ka
            slots.append([(a, ranges[a], ka), (b, ranges[b], kb)])
        return T, slots

    TF = per_core // F  # full tiles (32)
    T = TF + 1
    slots = []
    for a, b in pairs:
        na, nb = sizes[a], sizes[b]
        (sa, ea), (sb, eb) = ranges[a], ranges[b]
        r = na % F
        if r == 0:
            ta = na // F
            a_seg = [(sa, ea)]
            b_seg = [(sb, eb)]
            h_seg = [("dup", sb, HALF)]
            a_t = list(range(ta))
            b_t = list(range(ta, TF)) + [TF]
        elif r <= HALF:
            ta = na // F
            a_seg = [(sa, sa + ta * F)]
            b_seg = [(sb, eb), ("dup", sb, r)]
            h_seg = [(sa + ta * F, ea), ("dup", sa, HALF - r)]
            a_t = list(range(ta)) + [TF]
            b_t = list(range(ta, TF))
        else:
            rb = F - r
            ta1 = na // F + 1
            a_seg = [(sa, ea), ("dup", sa, rb)]
            b_seg = [(sb, eb - rb)]
            h_seg = [(eb - rb, eb), ("dup", sb, HALF - rb)]
            a_t = list(range(ta1))
            b_t = list(range(ta1, TF)) + [TF]
        # core xs = a_seg cols + b_seg cols + h_seg cols (= TF*F + HALF)
        slots.append([(a, a_seg, a_t), (b, b_seg, b_t), (None, h_seg, None)])
    return T, slots


def _core_inputs(x, T, core_slots, weights):
    """Build the per-core input dict (xs + masks); weights are shared."""
    from ml_dtypes import bfloat16

    if not _EXACT[0]:
        xs = np.empty((3, T * F), np.float32)
        mneg = np.full((1, 2 * T), NEG, np.float32)
        m01 = np.zeros((1, 2 * T), np.float32)
        t0 = 0
        for slot, (cid, (s, e), ktiles) in enumerate(core_slots):
            n = e - s
            cols = x[:, s:e] if n > 0 else x[:, :1]
            n = max(n, 1)
            pad = ktiles * F - n
            if pad > 0:
                cols = np.concatenate(
                    [cols, np.repeat(cols[:, :1], pad, axis=1)], axis=1
                )
            xs[:, t0 * F : (t0 + ktiles) * F] = cols
            mneg[0, slot * T + t0 : slot * T + t0 + ktiles] = 0.0
            m01[0, slot * T + t0 : slot * T + t0 + ktiles] = 1.0
            t0 += ktiles
    else:
        parts = []
        mneg = np.full((1, 2 * T), NEG, np.float32)
        m01 = np.zeros((1, 2 * T), np.float32)
        for slot, (cid, segs, tiles) in enumerate(core_slots):
            for seg in segs:
                if seg[0] == "dup":
                    _, at, cnt = seg
                    parts.append(np.repeat(x[:, at : at + 1], cnt, axis=1))
                else:
                    lo, hi = seg
                    parts.append(x[:, lo:hi])
            if tiles is not None:
                for t in tiles:
                    mneg[0, slot * T + t] = 0.0
                    m01[0, slot * T + t] = 1.0
        xs = np.concatenate(parts, axis=1)
        assert xs.shape[1] == (T - 1) * F + HALF, xs.shape
    weights = dict(weights)
    rowcat = np.concatenate([weights.pop("rowpre"), mneg], axis=1)
    return dict(
        xs=xs.astype(bfloat16),
        rowcat=np.ascontiguousarray(rowcat).astype(bfloat16),
        m01=m01.astype(bfloat16),
        **weights,
    )


def _build_nc(T):
    """Build + compile the SPMD Bass program for a per-core tile count T.

    Single-compute structure: pass 1 runs layers 1-2 once per tile, stashing
    f (bf16) in SBUF and per-tile maxes Mt; the combine turns Mt into the
    per-tile L3 bias table Cb; pass 2 reads the stashed f for layers 3-4.
    All matmul operands are bf16 (full-rate streaming + fast weight load);
    PSUM accumulation stays fp32, so only operand quantization is lost.
    """
    import concourse.mybir as mybir
    import concourse.tile as tile
    from concourse import bacc

    f32 = mybir.dt.float32
    f32r = mybir.dt.float32r
    bf16 = mybir.dt.bfloat16
    AF = mybir.ActivationFunctionType
    AX = mybir.AxisListType.X

    nc = bacc.Bacc("TRN2", target_bir_lowering=False, debug=False, num_devices=NCORES)

    last_w = HALF if _EXACT[0] else F
    XC = (T - 1) * F + last_w

    def w_of(t):
        return last_w if t == T - 1 else F

    xs_d = nc.dram_tensor("xs", [3, XC], bf16, kind="ExternalInput")
    w1t_d = nc.dram_tensor("w1t", [3, 128], bf16, kind="ExternalInput")
    w2t_d = nc.dram_tensor("w2t", [128, 256], bf16, kind="ExternalInput")
    w3gt_d = nc.dram_tensor("w3gt", [128, 2, 512], bf16, kind="ExternalInput")
    w32t_d = nc.dram_tensor("w32t", [128, 512], bf16, kind="ExternalInput")
    w4t_d = nc.dram_tensor("w4t", [128, 4, 1024], bf16, kind="ExternalInput")
    bc_d = nc.dram_tensor("bcat", [128, 3], f32, kind="ExternalInput")
    rc_d = nc.dram_tensor("rowcat", [1, 640 + 2 * T], bf16, kind="ExternalInput")
    m01_d = nc.dram_tensor("m01", [1, 2 * T], bf16, kind="ExternalInput")
    vt_d = nc.dram_tensor("vt", [128, T, 8], f32, kind="ExternalOutput")

    # first x chunk tiny so tile 0 can start early; rest split for overlap
    bounds = [0, F, 2 * F, 4 * F]
    step = max(F, (XC - 4 * F + 3) // 4)
    while bounds[-1] < XC:
        bounds.append(min(XC, bounds[-1] + step))

    with tile.TileContext(nc) as tc:
        with (
            tc.tile_pool(name="const", bufs=1) as cp,
            tc.tile_pool(name="work", bufs=4) as wp,
            tc.tile_pool(name="h3p", bufs=3) as h3p,
        ):
            xs = cp.tile([3, XC], bf16)
            w1t = cp.tile([3, 128], bf16)
            w2t = cp.tile([128, 256], bf16)
            w3gt = cp.tile([128, 2, 512], bf16)
            w32t = cp.tile([128, 512], bf16)
            w4t = cp.tile([128, 4, 1024], bf16)
            bc = cp.tile([128, 3], f32)
            rc = cp.tile([1, 640 + 2 * T], bf16)
            m01t = cp.tile([1, 2 * T], bf16)
            b1 = bc[:, 0:1]
            b2 = bc[:, 1:3]
            b3r = rc[:, 0:512]
            ones = rc[:, 512:640]
            mneg = rc[:, 640 : 640 + 2 * T]
            m01 = m01t[:]
            hsb = cp.tile([128, T, F], bf16)  # stashed layer-1 out h
            Mt = cp.tile([128, T, 2], f32)
            gk = cp.tile([128, 2, 2], bf16)
            cT0 = cp.tile([1, 512], bf16)
            cT1 = cp.tile([1, 512], bf16)
            Cb = cp.tile([128, 4, T], f32)
            V = cp.tile([128, T, 8], f32)

            # x chunks on gpsimd (SWDGE) run parallel to weights on sync
            # (HWDGE); pass-1-critical tensors first on each queue.
            nc.sync.dma_start(xs[:, 0 : bounds[1]], xs_d.ap()[:, 0 : bounds[1]])
            for a, b_ in zip(bounds[1:], bounds[2:]):
                nc.gpsimd.dma_start(xs[:, a:b_], xs_d.ap()[:, a:b_])
            nc.scalar.dma_start(w1t[:], w1t_d.ap())
            nc.scalar.dma_start(bc[:], bc_d.ap())
            nc.sync.dma_start(w2t[:], w2t_d.ap())
            nc.sync.dma_start(rc[:], rc_d.ap())
            nc.sync.dma_start(m01t[:], m01_d.ap())
            for t_sb, t_dr in ((w3gt, w3gt_d), (w32t, w32t_d), (w4t, w4t_d)):
                nc.sync.dma_start(t_sb[:], t_dr.ap())

            # pre-warm ACT function tables while the DMAs stream in
            warm = wp.tile([128, 1], f32, tag="gtmp")
            nc.gpsimd.memset(warm[:], 0.0)
            nc.scalar.activation(warm[:], warm[:], AF.Relu, bias=warm[:])
            nc.scalar.activation(warm[:], warm[:], AF.Identity, bias=warm[:])

            # ---- single psum scope for the whole kernel ------------------
            # tag "pa": [128, 2, F] 2-bank tiles, bufs=3 (pass-1 L2 pairs and
            # pass-2 L4 pairs); tag "pb": [128, F] 1-bank tiles, bufs=2
            # (L1, combine smalls, pass-2 L3). 6 + 2 = 8 banks.
            LOOK = 2
            with tc.tile_pool(name="psum", bufs=1, space="PSUM") as pp:

                def l1(u):
                    wu = w_of(u)
                    ph = pp.tile([128, wu], f32, tag="pb", bufs=2,
                                 name=f"p1h_{u}")
                    nc.tensor.matmul(
                        ph[:], w1t[:], xs[:, u * F : u * F + wu],
                        start=True, stop=True,
                    )
                    nc.scalar.activation(
                        hsb[:, u, 0:wu], ph[:], AF.Relu, bias=b1
                    )

                for u in range(min(LOOK, T)):
                    l1(u)
                for t in range(T):
                    if t + LOOK < T:
                        l1(t + LOOK)
                    wt = w_of(t)
                    pf = pp.tile([128, 2, F], f32, tag="pa", bufs=3,
                                 name=f"pf_{t}")
                    for m in range(2):
                        nc.tensor.matmul(
                            pf[:, m, 0:wt], w2t[:, m * 128 : (m + 1) * 128],
                            hsb[:, t, 0:wt], start=True, stop=True,
                        )
                    nc.vector.reduce_max(Mt[:, t, 0:2], pf[:, :, 0:wt], axis=AX)

                # ---- combine, interleaved with early pass-2 L3 matmuls ----
                # L3 psums live in "pa" pairs (mo01 / mo23) so the early-L3
                # matmuls never borrow the combine's "pb" rotation (which
                # would deadlock the in-order PE queue against Cb).
                def l3mms(t, half):
                    wt = w_of(t)
                    p3 = pp.tile([128, 2, F], f32, tag="pa", bufs=3,
                                 name=f"p3_{t}_{half}")
                    for sub in range(2):
                        mo = 2 * half + sub
                        nc.tensor.matmul(
                            p3[:, sub, 0:wt], w32t[:, mo * 128 : (mo + 1) * 128],
                            hsb[:, t, 0:wt], start=True, stop=True,
                        )
                    return p3

                def l3acts(t, h3, half, p3):
                    wt = w_of(t)
                    for sub in range(2):
                        mo = 2 * half + sub
                        nc.scalar.activation(
                            h3[:, mo, 0:wt], p3[:, sub, 0:wt], AF.Relu,
                            bias=Cb[:, mo, t : t + 1],
                        )

                def l3pair(t, h3, half):
                    l3acts(t, h3, half, l3mms(t, half))

                h3s = {0: h3p.tile([128, 4, F], bf16, tag="h3", name="h3_0"),
                       1: h3p.tile([128, 4, F], bf16, tag="h3", name="h3_1")}

                for m in range(2):
                    pmask = pp.tile([128, 2 * T], f32, tag="pb", bufs=2)
                    nc.tensor.matmul(pmask[:], ones, mneg, start=True, stop=True)
                    cmb = wp.tile([128, 2 * T], f32, tag="cmb")
                    for sl in range(2):
                        nc.vector.tensor_add(
                            cmb[:, sl * T : (sl + 1) * T], Mt[:, :, m],
                            pmask[:, sl * T : (sl + 1) * T],
                        )
                    for sl in range(2):
                        gtmp = wp.tile([128, 1], f32, tag="gtmp")
                        nc.vector.reduce_max(
                            gtmp[:], cmb[:, sl * T : (sl + 1) * T], axis=AX
                        )
                        nc.vector.tensor_add(
                            gk[:, m, sl : sl + 1], gtmp[:], b2[:, m : m + 1]
                        )

                # independent early-L3 matmuls keep the PE busy while the
                # DVE finishes the g chain; their ACTs (which read Cb) are
                # emitted after the Cb writes below
                early = [(0, 0, l3mms(0, 0)), (0, 1, l3mms(0, 1)),
                         (1, 0, l3mms(1, 0))]

                # c rows live on partition 0 so outer-product lhsT is legal;
                # the b3 row folds into the DVE copy instead of a third matmul
                for sl, cTs in ((0, cT0), (1, cT1)):
                    pcT = pp.tile([1, 512], f32, tag="pb", bufs=2)
                    nc.tensor.matmul(
                        pcT[:], gk[:, 0, sl : sl + 1], w3gt[:, 0, :],
                        start=True, stop=False,
                    )
                    nc.tensor.matmul(
                        pcT[:], gk[:, 1, sl : sl + 1], w3gt[:, 1, :],
                        start=False, stop=True,
                    )
                    nc.vector.tensor_add(cTs[:], pcT[:], b3r)

                for mo in range(4):
                    pC = pp.tile([128, T], f32, tag="pb", bufs=2)
                    nc.tensor.matmul(
                        pC[:], cT0[:, mo * 128 : (mo + 1) * 128], m01[:, 0:T],
                        start=True, stop=False,
                    )
                    nc.tensor.matmul(
                        pC[:], cT1[:, mo * 128 : (mo + 1) * 128],
                        m01[:, T : 2 * T], start=False, stop=True,
                    )
                    nc.vector.tensor_copy(Cb[:, mo, :], pC[:])

                for t_, half_, p3_ in early:
                    l3acts(t_, h3s[t_], half_, p3_)
                l3pair(1, h3s[1], 1)

                # ---- pass 2: L4 of tile t with L3 halves of t+1 / t+2
                # interleaved between the L4 pair streams.
                for t in range(T):
                    h3cur = h3s.pop(t)
                    wt = w_of(t)
                    for pair in range(4):
                        p4 = pp.tile([128, 2, F], f32, tag="pa", bufs=3,
                                     name=f"p4_{t}_{pair}")
                        for sub in range(2):
                            mo = 2 * pair + sub
                            for k in range(4):
                                nc.tensor.matmul(
                                    p4[:, sub, 0:wt],
                                    w4t[:, k, mo * 128 : (mo + 1) * 128],
                                    h3cur[:, k, 0:wt], start=(k == 0), stop=(k == 3),
                                )
                        nc.vector.reduce_max(
                            V[:, t, 2 * pair : 2 * pair + 2], p4[:, :, 0:wt],
                            axis=AX,
                        )
                        if pair == 0 and t + 2 < T:
                            h3s[t + 2] = h3p.tile([128, 4, F], bf16, tag="h3",
                                                  name=f"h3_{t + 2}")
                            l3pair(t + 2, h3s[t + 2], 0)
                        elif pair == 2 and t + 2 < T:
                            l3pair(t + 2, h3s[t + 2], 1)
                    if t % 2 == 1 or t == T - 1:
                        a = (t // 2) * 2
                        nc.sync.dma_start(
                            vt_d.ap()[:, a : t + 1, :], V[:, a : t + 1, :]
                        )

    nc.compile()
    return nc


def _prep(x, npts, W1, b1, g1, be1, m1, v1, W2, b2, g2, be2, m2, v2,
          W3, b3, g3, be3, m3, v3, W4, b4, g4, be4, m4, v4):
    """Host-side preprocessing shared by kernel() and the test harness."""
    W1f, b1f = _fold_bn(W1, b1, g1, be1, m1, v1)
    W2f, b2f = _fold_bn(W2, b2, g2, be2, m2, v2)
    W3f_, b3f = _fold_bn(W3, b3, g3, be3, m3, v3)
    W4f, b4f = _fold_bn(W4, b4, g4, be4, m4, v4)

    from ml_dtypes import bfloat16

    weights = dict(
        w1t=np.ascontiguousarray(W1f.T).astype(bfloat16),
        w2t=np.ascontiguousarray(W2f.T).astype(bfloat16),
        w3gt=np.ascontiguousarray(W3f_[:, :256].T.reshape(2, 128, 512).transpose(1, 0, 2)).astype(bfloat16),
        w32t=np.ascontiguousarray((W3f_[:, 256:] @ W2f).T).astype(bfloat16),
        w4t=np.ascontiguousarray(W4f.T.reshape(4, 128, 1024).transpose(1, 0, 2)).astype(bfloat16),
        bcat=np.ascontiguousarray(
            np.concatenate([b1f[:, None], b2f.reshape(2, 128).T], axis=1)
        ),
        # fsb holds raw W2'h (no b2): fold W3f @ b2 into the b3 row
        rowpre=np.concatenate(
            [(b3f + W3f_[:, 256:] @ b2f)[None, :].astype(np.float32),
             np.ones((1, 128), np.float32)], axis=1
        ),
    )

    T, slots = _plan(npts)
    x = np.asarray(x, np.float32)
    in_maps = [_core_inputs(x, T, slots, weights) for slots in slots]
    return T, slots, in_maps, b4f


def _gather(results, T, slots, b4f):
    """Combine per-core per-tile maxes into the [B, 1024] output."""
    out = np.empty((B, 1024), np.float32)
    for c, core_slots in enumerate(slots):
        vt = results[c]["vt"]  # [128, T, 8]; channel = mo*128 + partition
        chan = vt.transpose(2, 0, 1).reshape(1024, T)
        if not _EXACT[0]:
            t0 = 0
            for cid, _rng, ktiles in core_slots:
                out[cid] = chan[:, t0 : t0 + ktiles].max(axis=1) + b4f
                t0 += ktiles
        else:
            for cid, _segs, tiles in core_slots:
                if cid is None:
                    continue
                out[cid] = chan[:, tiles].max(axis=1) + b4f
    return out


def kernel(**inputs):
    from concourse.bass_utils import run_bass_kernel_spmd

    # force host numpy: jax arrays would route host math through the (axon)
    # device backend
    inputs = {k: np.asarray(v) for k, v in inputs.items()}
    T, slots, in_maps, b4f = _prep(**inputs)
    nc = _build_nc(T)
    res = run_bass_kernel_spmd(nc, in_maps, core_ids=list(range(NCORES)))
    return _gather(res.results, T, slots, b4f)



# revision 31
# speedup vs baseline: 1.1951x; 1.1951x over previous
"""Trainium2 Bass kernel for nn_Encoder_48017734369665 (PointNet-style
segment-reduce encoder).

Network (B=16 clouds, N=131072 points, ragged via npts):
    h  = relu(bn1(W1 @ x))            [128, N]
    f  = bn2(W2 @ h)                  [256, N]
    g  = segment_max(f)               [B, 256]
    h3 = relu(bn3(W3 @ [g[seg]; f]))  [512, N]
    h4 = bn4(W4 @ h3)                 [1024, N]
    out = segment_max(h4)             [B, 1024]

Strategy:
  * Inference-mode batchnorms fold into the affine layers on the host; the
    f-path of layer 3 folds further: W32 = W3f @ W2 lets layer 3 contract
    directly over h (K=128), so f itself is never materialized for layer 3.
  * All matmul operands are bf16 (full-rate 1 col/cycle streaming plus fast
    weight load, which fp32 operands do not get); PSUM accumulation stays
    fp32. Measured end-to-end error ~5e-3 vs the 2e-2 gate. fp8 DoubleRow
    was evaluated and rejected: e4m3 quantization of W4/h3 alone measures
    4e-2 — over the gate.
  * Each core owns 2 whole clouds, paired so their sizes sum to exactly
    N/8 = 16384 points (true for this npts distribution). The core runs 32
    full 512-point tiles plus one 256-wide half-tile holding both clouds'
    sub-tile remainders (each side padded with duplicates of its own
    points, which is max-invariant); slot-to-cloud assignment rides in the
    per-core mask data, so the SPMD program is identical on every core and
    no full padding tile is ever computed.
  * Pass 1 per tile: L1 -> h (bf16, stashed in SBUF for the whole core),
    L2 pair into PSUM, one DVE reduce -> per-tile maxes Mt. The combine
    masks Mt per cloud slot (SPMD-safe for any per-core cloud split),
    builds g, then c = W3g' g + b3' + W3f' b2', and broadcasts c into the
    per-tile bias table Cb via tiny K=1 matmuls.
  * Pass 2 per tile: L3 = W32 @ h (4 matmuls) -> relu(. + Cb) -> h3 (bf16),
    L4 = 32 matmuls into 2-bank PSUM pairs, one DVE reduce per pair ->
    per-tile maxes V, DMA'd out every 2 tiles. Host combines per-tile maxes
    into [B, 1024] (adding b4'): no cross-device communication at all.
  * One PSUM pool scope with two bank tags ([128,2,F]x3 shared by the L2
    pairs and L4 pairs; [128,F]x2 for L1/combine/L3) lets L3 matmuls of the
    first tiles interleave into the combine's DVE window; L3 of tile t+2 is
    emitted in halves between L4 pair streams of tile t, keeping the PE
    gap-free at the 216 ns/matmul bf16 roofline through all of pass 2.
"""

import numpy as np

EPS = 1e-5
B = 16
N = 131072
F = 512  # points per tile (fp32 moving-operand / PSUM-bank limit)
NCORES = 8
NEG = -1.0e30


def _fold_bn(W, b, g, be, m, v):
    """bn(W@x + b) == W' @ x + b' with W' = s*W, b' = s*(b-m)+be, s=g/sqrt(v+eps)."""
    s = g / np.sqrt(v + EPS)
    return (s[:, None] * W).astype(np.float32), (s * (b - m) + be).astype(np.float32)


def _cloud_ranges(npts):
    """Per-cloud [start, end) column ranges exactly as the reference's
    jnp.repeat(..., total_repeat_length=N) maps points to clouds: truncate
    if sum > N, extend the last cloud if sum < N."""
    npts = np.maximum(np.asarray(npts, np.int64), 0)
    ends = np.minimum(np.cumsum(npts), N)
    starts = np.concatenate([[0], ends[:-1]])
    ends = ends.copy()
    ends[-1] = N  # pad semantics: trailing points belong to the last cloud
    return [(int(s), int(e)) for s, e in zip(starts, ends)]


HALF = 256  # width of the shared remainder tile (tile T-1) in exact mode
_EXACT = [False]


def _plan(npts):
    """Pair clouds 2-per-core. In exact mode (each pair sums to N/NCORES
    points, true for this problem's npts), a core runs 32 full tiles plus
    one HALF-wide tile that holds both clouds' sub-tile remainders (each
    padded with duplicates of its own points, which is max-invariant).

    Returns (T, slots): slots[c] = [(cid, col_segments, tile_list)] where
    col_segments rebuild the core's xs and tile_list are the Mt/V slots
    owned by that cloud.
    """
    ranges = _cloud_ranges(npts)
    sizes = [e - s for s, e in ranges]
    order = np.argsort(sizes)[::-1]
    pairs = [(int(order[i]), int(order[2 * NCORES - 1 - i])) for i in range(NCORES)]
    per_core = N // NCORES
    _EXACT[0] = all(sizes[a] + sizes[b] == per_core for a, b in pairs)
    if not _EXACT[0]:
        # fallback: per-cloud whole tiles (original scheme)
        ktiles = [max(1, -(-sizes[b] // F)) for b in range(2 * NCORES)]
        T = max(ktiles[a] + ktiles[b] for a, b in pairs)
        slots = []
        for a, b in pairs:
            ka = ktiles[a]
            kb = T - ka
            slots.append([(a, ranges[a], ka), (b, ranges[b], kb)])
        return T, slots

    TF = per_core // F  # full tiles (32)
    T = TF + 1
    slots = []
    for a, b in pairs:
        na, nb = sizes[a], sizes[b]
        (sa, ea), (sb, eb) = ranges[a], ranges[b]
        r = na % F
        if r == 0:
            ta = na // F
            a_seg = [(sa, ea)]
            b_seg = [(sb, eb)]
            h_seg = [("dup", sb, HALF)]
            a_t = list(range(ta))
            b_t = list(range(ta, TF)) + [TF]
        elif r <= HALF:
            ta = na // F
            a_seg = [(sa, sa + ta * F)]
            b_seg = [(sb, eb), ("dup", sb, r)]
            h_seg = [(sa + ta * F, ea), ("dup", sa, HALF - r)]
            a_t = list(range(ta)) + [TF]
            b_t = list(range(ta, TF))
        else:
            rb = F - r
            ta1 = na // F + 1
            a_seg = [(sa, ea), ("dup", sa, rb)]
            b_seg = [(sb, eb - rb)]
            h_seg = [(eb - rb, eb), ("dup", sb, HALF - rb)]
            a_t = list(range(ta1))
            b_t = list(range(ta1, TF)) + [TF]
        # core xs = a_seg cols + b_seg cols + h_seg cols (= TF*F + HALF)
        slots.append([(a, a_seg, a_t), (b, b_seg, b_t), (None, h_seg, None)])
    return T, slots


def _core_inputs(x, T, core_slots, weights):
    """Build the per-core input dict (xs + masks); weights are shared."""
    from ml_dtypes import bfloat16

    if not _EXACT[0]:
        xs = np.empty((3, T * F), np.float32)
        mneg = np.full((1, 2 * T), NEG, np.float32)
        m01 = np.zeros((1, 2 * T), np.float32)
        t0 = 0
        for slot, (cid, (s, e), ktiles) in enumerate(core_slots):
            n = e - s
            cols = x[:, s:e] if n > 0 else x[:, :1]
            n = max(n, 1)
            pad = ktiles * F - n
            if pad > 0:
                cols = np.concatenate(
                    [cols, np.repeat(cols[:, :1], pad, axis=1)], axis=1
                )
            xs[:, t0 * F : (t0 + ktiles) * F] = cols
            mneg[0, slot * T + t0 : slot * T + t0 + ktiles] = 0.0
            m01[0, slot * T + t0 : slot * T + t0 + ktiles] = 1.0
            t0 += ktiles
    else:
        parts = []
        mneg = np.full((1, 2 * T), NEG, np.float32)
        m01 = np.zeros((1, 2 * T), np.float32)
        for slot, (cid, segs, tiles) in enumerate(core_slots):
            for seg in segs:
                if seg[0] == "dup":
                    _, at, cnt = seg
                    parts.append(np.repeat(x[:, at : at + 1], cnt, axis=1))
                else:
                    lo, hi = seg
                    parts.append(x[:, lo:hi])
            if tiles is not None:
                for t in tiles:
                    mneg[0, slot * T + t] = 0.0
                    m01[0, slot * T + t] = 1.0
        xs = np.concatenate(parts, axis=1)
        assert xs.shape[1] == (T - 1) * F + HALF, xs.shape
    weights = dict(weights)
    rowcat = np.concatenate([weights.pop("rowpre"), mneg], axis=1)
    return dict(
        xs=xs.astype(bfloat16),
        rowcat=np.ascontiguousarray(rowcat).astype(bfloat16),
        m01=m01.astype(bfloat16),
        **weights,
    )


def _build_nc(T):
    """Build + compile the SPMD Bass program for a per-core tile count T.

    Single-compute structure: pass 1 runs layers 1-2 once per tile, stashing
    f (bf16) in SBUF and per-tile maxes Mt; the combine turns Mt into the
    per-tile L3 bias table Cb; pass 2 reads the stashed f for layers 3-4.
    All matmul operands are bf16 (full-rate streaming + fast weight load);
    PSUM accumulation stays fp32, so only operand quantization is lost.
    """
    import concourse.mybir as mybir
    import concourse.tile as tile
    from concourse import bacc

    f32 = mybir.dt.float32
    f32r = mybir.dt.float32r
    bf16 = mybir.dt.bfloat16
    AF = mybir.ActivationFunctionType
    AX = mybir.AxisListType.X

    nc = bacc.Bacc("TRN2", target_bir_lowering=False, debug=False, num_devices=NCORES)

    last_w = HALF if _EXACT[0] else F
    XC = (T - 1) * F + last_w

    def w_of(t):
        return last_w if t == T - 1 else F

    xs_d = nc.dram_tensor("xs", [3, XC], bf16, kind="ExternalInput")
    w1t_d = nc.dram_tensor("w1t", [3, 128], bf16, kind="ExternalInput")
    w2t_d = nc.dram_tensor("w2t", [128, 256], bf16, kind="ExternalInput")
    w3gt_d = nc.dram_tensor("w3gt", [128, 2, 512], bf16, kind="ExternalInput")
    w32t_d = nc.dram_tensor("w32t", [128, 512], bf16, kind="ExternalInput")
    w4t_d = nc.dram_tensor("w4t", [128, 4, 1024], bf16, kind="ExternalInput")
    bc_d = nc.dram_tensor("bcat", [128, 3], f32, kind="ExternalInput")
    rc_d = nc.dram_tensor("rowcat", [1, 640 + 2 * T], bf16, kind="ExternalInput")
    m01_d = nc.dram_tensor("m01", [1, 2 * T], bf16, kind="ExternalInput")
    vt_d = nc.dram_tensor("vt", [128, T, 8], f32, kind="ExternalOutput")

    # first x chunk tiny so tile 0 can start early; rest split for overlap
    bounds = [0, F, 2 * F, 4 * F]
    step = max(F, (XC - 4 * F + 3) // 4)
    while bounds[-1] < XC:
        bounds.append(min(XC, bounds[-1] + step))

    with tile.TileContext(nc) as tc:
        with (
            tc.tile_pool(name="const", bufs=1) as cp,
            tc.tile_pool(name="work", bufs=4) as wp,
            tc.tile_pool(name="h3p", bufs=3) as h3p,
        ):
            xs = cp.tile([3, XC], bf16)
            w1t = cp.tile([3, 128], bf16)
            w2t = cp.tile([128, 256], bf16)
            w3gt = cp.tile([128, 2, 512], bf16)
            w32t = cp.tile([128, 512], bf16)
            w4t = cp.tile([128, 4, 1024], bf16)
            bc = cp.tile([128, 3], f32)
            rc = cp.tile([1, 640 + 2 * T], bf16)
            m01t = cp.tile([1, 2 * T], bf16)
            b1 = bc[:, 0:1]
            b2 = bc[:, 1:3]
            b3r = rc[:, 0:512]
            ones = rc[:, 512:640]
            mneg = rc[:, 640 : 640 + 2 * T]
            m01 = m01t[:]
            hsb = cp.tile([128, T, F], bf16)  # stashed layer-1 out h
            Mt = cp.tile([128, T, 2], f32)
            gk = cp.tile([128, 2, 2], bf16)
            cT0 = cp.tile([1, 512], bf16)
            cT1 = cp.tile([1, 512], bf16)
            Cb = cp.tile([128, 4, T], f32)
            V = cp.tile([128, T, 8], f32)

            # x chunks on gpsimd (SWDGE) run parallel to weights on sync
            # (HWDGE); pass-1-critical tensors first on each queue.
            nc.sync.dma_start(xs[:, 0 : bounds[1]], xs_d.ap()[:, 0 : bounds[1]])
            for a, b_ in zip(bounds[1:], bounds[2:]):
                nc.gpsimd.dma_start(xs[:, a:b_], xs_d.ap()[:, a:b_])
            nc.scalar.dma_start(w1t[:], w1t_d.ap())
            nc.scalar.dma_start(bc[:], bc_d.ap())
            nc.sync.dma_start(w2t[:], w2t_d.ap())
            nc.sync.dma_start(rc[:], rc_d.ap())
            nc.sync.dma_start(m01t[:], m01_d.ap())
            for t_sb, t_dr in ((w3gt, w3gt_d), (w32t, w32t_d), (w4t, w4t_d)):
                nc.sync.dma_start(t_sb[:], t_dr.ap())

            # pre-warm ACT function tables while the DMAs stream in
            warm = wp.tile([128, 1], f32, tag="gtmp")
            nc.gpsimd.memset(warm[:], 0.0)
            nc.scalar.activation(warm[:], warm[:], AF.Relu, bias=warm[:])
            nc.scalar.activation(warm[:], warm[:], AF.Identity, bias=warm[:])

            # ---- single psum scope for the whole kernel ------------------
            # tag "pa": [128, 2, F] 2-bank tiles, bufs=3 (pass-1 L2 pairs and
            # pass-2 L4 pairs); tag "pb": [128, F] 1-bank tiles, bufs=2
            # (L1, combine smalls, pass-2 L3). 6 + 2 = 8 banks.
            LOOK = 2
            with tc.tile_pool(name="psum", bufs=1, space="PSUM") as pp:

                def l1(u):
                    wu = w_of(u)
                    ph = pp.tile([128, wu], f32, tag="pb", bufs=2,
                                 name=f"p1h_{u}")
                    nc.tensor.matmul(
                        ph[:], w1t[:], xs[:, u * F : u * F + wu],
                        start=True, stop=True,
                    )
                    nc.scalar.activation(
                        hsb[:, u, 0:wu], ph[:], AF.Relu, bias=b1
                    )

                for u in range(min(LOOK, T)):
                    l1(u)
                for t in range(T):
                    if t + LOOK < T:
                        l1(t + LOOK)
                    wt = w_of(t)
                    pf = pp.tile([128, 2, F], f32, tag="pa", bufs=3,
                                 name=f"pf_{t}")
                    for m in range(2):
                        nc.tensor.matmul(
                            pf[:, m, 0:wt], w2t[:, m * 128 : (m + 1) * 128],
                            hsb[:, t, 0:wt], start=True, stop=True,
                        )
                    nc.vector.reduce_max(Mt[:, t, 0:2], pf[:, :, 0:wt], axis=AX)

                # ---- combine, interleaved with early pass-2 L3 matmuls ----
                # L3 psums live in "pa" pairs (mo01 / mo23) so the early-L3
                # matmuls never borrow the combine's "pb" rotation (which
                # would deadlock the in-order PE queue against Cb).
                def l3mms(t, half):
                    wt = w_of(t)
                    p3 = pp.tile([128, 2, F], f32, tag="pa", bufs=3,
                                 name=f"p3_{t}_{half}")
                    for sub in range(2):
                        mo = 2 * half + sub
                        nc.tensor.matmul(
                            p3[:, sub, 0:wt], w32t[:, mo * 128 : (mo + 1) * 128],
                            hsb[:, t, 0:wt], start=True, stop=True,
                        )
                    return p3

                def l3acts(t, h3, half, p3):
                    wt = w_of(t)
                    for sub in range(2):
                        mo = 2 * half + sub
                        nc.scalar.activation(
                            h3[:, mo, 0:wt], p3[:, sub, 0:wt], AF.Relu,
                            bias=Cb[:, mo, t : t + 1],
                        )

                def l3pair(t, h3, half):
                    l3acts(t, h3, half, l3mms(t, half))

                h3s = {0: h3p.tile([128, 4, F], bf16, tag="h3", name="h3_0"),
                       1: h3p.tile([128, 4, F], bf16, tag="h3", name="h3_1")}

                for m in range(2):
                    pmask = pp.tile([128, 2 * T], f32, tag="pb", bufs=2)
                    nc.tensor.matmul(pmask[:], ones, mneg, start=True, stop=True)
                    cmb = wp.tile([128, 2 * T], f32, tag="cmb")
                    for sl in range(2):
                        nc.vector.tensor_add(
                            cmb[:, sl * T : (sl + 1) * T], Mt[:, :, m],
                            pmask[:, sl * T : (sl + 1) * T],
                        )
                    for sl in range(2):
                        gtmp = wp.tile([128, 1], f32, tag="gtmp")
                        nc.vector.reduce_max(
                            gtmp[:], cmb[:, sl * T : (sl + 1) * T], axis=AX
                        )
                        nc.vector.tensor_add(
                            gk[:, m, sl : sl + 1], gtmp[:], b2[:, m : m + 1]
                        )

                # independent early-L3 matmuls keep the PE busy while the
                # DVE finishes the g chain; their ACTs (which read Cb) are
                # emitted after the Cb writes below
                early = [(0, 0, l3mms(0, 0)), (0, 1, l3mms(0, 1)),
                         (1, 0, l3mms(1, 0))]

                # c rows live on partition 0 so outer-product lhsT is legal
                for sl, cTs in ((0, cT0), (1, cT1)):
                    pcT = pp.tile([1, 512], f32, tag="pb", bufs=2)
                    nc.tensor.matmul(
                        pcT[:], gk[:, 0, sl : sl + 1], w3gt[:, 0, :],
                        start=True, stop=False,
                    )
                    nc.tensor.matmul(
                        pcT[:], gk[:, 1, sl : sl + 1], w3gt[:, 1, :],
                        start=False, stop=False,
                    )
                    nc.tensor.matmul(
                        pcT[:], ones[:, 0:1], b3r, start=False, stop=True
                    )
                    nc.vector.tensor_copy(cTs[:], pcT[:])

                for mo in range(4):
                    pC = pp.tile([128, T], f32, tag="pb", bufs=2)
                    nc.tensor.matmul(
                        pC[:], cT0[:, mo * 128 : (mo + 1) * 128], m01[:, 0:T],
                        start=True, stop=False,
                    )
                    nc.tensor.matmul(
                        pC[:], cT1[:, mo * 128 : (mo + 1) * 128],
                        m01[:, T : 2 * T], start=False, stop=True,
                    )
                    nc.vector.tensor_copy(Cb[:, mo, :], pC[:])

                for t_, half_, p3_ in early:
                    l3acts(t_, h3s[t_], half_, p3_)
                l3pair(1, h3s[1], 1)

                # ---- pass 2: L4 of tile t with L3 halves of t+1 / t+2
                # interleaved between the L4 pair streams.
                for t in range(T):
                    h3cur = h3s.pop(t)
                    wt = w_of(t)
                    for pair in range(4):
                        p4 = pp.tile([128, 2, F], f32, tag="pa", bufs=3,
                                     name=f"p4_{t}_{pair}")
                        for sub in range(2):
                            mo = 2 * pair + sub
                            for k in range(4):
                                nc.tensor.matmul(
                                    p4[:, sub, 0:wt],
                                    w4t[:, k, mo * 128 : (mo + 1) * 128],
                                    h3cur[:, k, 0:wt], start=(k == 0), stop=(k == 3),
                                )
                        nc.vector.reduce_max(
                            V[:, t, 2 * pair : 2 * pair + 2], p4[:, :, 0:wt],
                            axis=AX,
                        )
                        if pair == 0 and t + 2 < T:
                            h3s[t + 2] = h3p.tile([128, 4, F], bf16, tag="h3",
                                                  name=f"h3_{t + 2}")
                            l3pair(t + 2, h3s[t + 2], 0)
                        elif pair == 2 and t + 2 < T:
                            l3pair(t + 2, h3s[t + 2], 1)
                    if t % 2 == 1 or t == T - 1:
                        a = (t // 2) * 2
                        nc.sync.dma_start(
                            vt_d.ap()[:, a : t + 1, :], V[:, a : t + 1, :]
                        )

    nc.compile()
    return nc


def _prep(x, npts, W1, b1, g1, be1, m1, v1, W2, b2, g2, be2, m2, v2,
          W3, b3, g3, be3, m3, v3, W4, b4, g4, be4, m4, v4):
    """Host-side preprocessing shared by kernel() and the test harness."""
    W1f, b1f = _fold_bn(W1, b1, g1, be1, m1, v1)
    W2f, b2f = _fold_bn(W2, b2, g2, be2, m2, v2)
    W3f_, b3f = _fold_bn(W3, b3, g3, be3, m3, v3)
    W4f, b4f = _fold_bn(W4, b4, g4, be4, m4, v4)

    from ml_dtypes import bfloat16

    weights = dict(
        w1t=np.ascontiguousarray(W1f.T).astype(bfloat16),
        w2t=np.ascontiguousarray(W2f.T).astype(bfloat16),
        w3gt=np.ascontiguousarray(W3f_[:, :256].T.reshape(2, 128, 512).transpose(1, 0, 2)).astype(bfloat16),
        w32t=np.ascontiguousarray((W3f_[:, 256:] @ W2f).T).astype(bfloat16),
        w4t=np.ascontiguousarray(W4f.T.reshape(4, 128, 1024).transpose(1, 0, 2)).astype(bfloat16),
        bcat=np.ascontiguousarray(
            np.concatenate([b1f[:, None], b2f.reshape(2, 128).T], axis=1)
        ),
        # fsb holds raw W2'h (no b2): fold W3f @ b2 into the b3 row
        rowpre=np.concatenate(
            [(b3f + W3f_[:, 256:] @ b2f)[None, :].astype(np.float32),
             np.ones((1, 128), np.float32)], axis=1
        ),
    )

    T, slots = _plan(npts)
    x = np.asarray(x, np.float32)
    in_maps = [_core_inputs(x, T, slots, weights) for slots in slots]
    return T, slots, in_maps, b4f


def _gather(results, T, slots, b4f):
    """Combine per-core per-tile maxes into the [B, 1024] output."""
    out = np.empty((B, 1024), np.float32)
    for c, core_slots in enumerate(slots):
        vt = results[c]["vt"]  # [128, T, 8]; channel = mo*128 + partition
        chan = vt.transpose(2, 0, 1).reshape(1024, T)
        if not _EXACT[0]:
            t0 = 0
            for cid, _rng, ktiles in core_slots:
                out[cid] = chan[:, t0 : t0 + ktiles].max(axis=1) + b4f
                t0 += ktiles
        else:
            for cid, _segs, tiles in core_slots:
                if cid is None:
                    continue
                out[cid] = chan[:, tiles].max(axis=1) + b4f
    return out


def kernel(**inputs):
    from concourse.bass_utils import run_bass_kernel_spmd

    # force host numpy: jax arrays would route host math through the (axon)
    # device backend
    inputs = {k: np.asarray(v) for k, v in inputs.items()}
    T, slots, in_maps, b4f = _prep(**inputs)
    nc = _build_nc(T)
    res = run_bass_kernel_spmd(nc, in_maps, core_ids=list(range(NCORES)))
    return _gather(res.results, T, slots, b4f)



# revision 32
# speedup vs baseline: 1.1972x; 1.0018x over previous
"""Trainium2 Bass kernel for nn_Encoder_48017734369665 (PointNet-style
segment-reduce encoder).

Network (B=16 clouds, N=131072 points, ragged via npts):
    h  = relu(bn1(W1 @ x))            [128, N]
    f  = bn2(W2 @ h)                  [256, N]
    g  = segment_max(f)               [B, 256]
    h3 = relu(bn3(W3 @ [g[seg]; f]))  [512, N]
    h4 = bn4(W4 @ h3)                 [1024, N]
    out = segment_max(h4)             [B, 1024]

Strategy:
  * Inference-mode batchnorms fold into the affine layers on the host; the
    f-path of layer 3 folds further: W32 = W3f @ W2 lets layer 3 contract
    directly over h (K=128), so f itself is never materialized for layer 3.
  * All matmul operands are bf16 (full-rate 1 col/cycle streaming plus fast
    weight load, which fp32 operands do not get); PSUM accumulation stays
    fp32. Measured end-to-end error ~5e-3 vs the 2e-2 gate. fp8 DoubleRow
    was evaluated and rejected: e4m3 quantization of W4/h3 alone measures
    4e-2 — over the gate.
  * Each core owns 2 whole clouds, paired so their sizes sum to exactly
    N/8 = 16384 points (true for this npts distribution). The core runs 32
    full 512-point tiles plus one 256-wide half-tile holding both clouds'
    sub-tile remainders (each side padded with duplicates of its own
    points, which is max-invariant); slot-to-cloud assignment rides in the
    per-core mask data, so the SPMD program is identical on every core and
    no full padding tile is ever computed.
  * Pass 1 per tile: L1 -> h (bf16, stashed in SBUF for the whole core),
    L2 pair into PSUM, one DVE reduce -> per-tile maxes Mt. The combine
    masks Mt per cloud slot (SPMD-safe for any per-core cloud split),
    builds g, then c = W3g' g + b3' + W3f' b2', and broadcasts c into the
    per-tile bias table Cb via tiny K=1 matmuls.
  * Pass 2 per tile: L3 = W32 @ h (4 matmuls) -> relu(. + Cb) -> h3 (bf16),
    L4 = 32 matmuls into 2-bank PSUM pairs, one DVE reduce per pair ->
    per-tile maxes V, DMA'd out every 2 tiles. Host combines per-tile maxes
    into [B, 1024] (adding b4'): no cross-device communication at all.
  * One PSUM pool scope with two bank tags ([128,2,F]x3 shared by the L2
    pairs and L4 pairs; [128,F]x2 for L1/combine/L3) lets L3 matmuls of the
    first tiles interleave into the combine's DVE window; L3 of tile t+2 is
    emitted in halves between L4 pair streams of tile t, keeping the PE
    gap-free at the 216 ns/matmul bf16 roofline through all of pass 2.
"""

import numpy as np

EPS = 1e-5
B = 16
N = 131072
F = 512  # points per tile (fp32 moving-operand / PSUM-bank limit)
NCORES = 8
NEG = -1.0e30


def _fold_bn(W, b, g, be, m, v):
    """bn(W@x + b) == W' @ x + b' with W' = s*W, b' = s*(b-m)+be, s=g/sqrt(v+eps)."""
    s = g / np.sqrt(v + EPS)
    return (s[:, None] * W).astype(np.float32), (s * (b - m) + be).astype(np.float32)


def _cloud_ranges(npts):
    """Per-cloud [start, end) column ranges exactly as the reference's
    jnp.repeat(..., total_repeat_length=N) maps points to clouds: truncate
    if sum > N, extend the last cloud if sum < N."""
    npts = np.maximum(np.asarray(npts, np.int64), 0)
    ends = np.minimum(np.cumsum(npts), N)
    starts = np.concatenate([[0], ends[:-1]])
    ends = ends.copy()
    ends[-1] = N  # pad semantics: trailing points belong to the last cloud
    return [(int(s), int(e)) for s, e in zip(starts, ends)]


HALF = 256  # width of the shared remainder tile (tile T-1) in exact mode
_EXACT = [False]


def _plan(npts):
    """Pair clouds 2-per-core. In exact mode (each pair sums to N/NCORES
    points, true for this problem's npts), a core runs 32 full tiles plus
    one HALF-wide tile that holds both clouds' sub-tile remainders (each
    padded with duplicates of its own points, which is max-invariant).

    Returns (T, slots): slots[c] = [(cid, col_segments, tile_list)] where
    col_segments rebuild the core's xs and tile_list are the Mt/V slots
    owned by that cloud.
    """
    ranges = _cloud_ranges(npts)
    sizes = [e - s for s, e in ranges]
    order = np.argsort(sizes)[::-1]
    pairs = [(int(order[i]), int(order[2 * NCORES - 1 - i])) for i in range(NCORES)]
    per_core = N // NCORES
    _EXACT[0] = all(sizes[a] + sizes[b] == per_core for a, b in pairs)
    if not _EXACT[0]:
        # fallback: per-cloud whole tiles (original scheme)
        ktiles = [max(1, -(-sizes[b] // F)) for b in range(2 * NCORES)]
        T = max(ktiles[a] + ktiles[b] for a, b in pairs)
        slots = []
        for a, b in pairs:
            ka = ktiles[a]
            kb = T - ka
            slots.append([(a, ranges[a], ka), (b, ranges[b], kb)])
        return T, slots

    TF = per_core // F  # full tiles (32)
    T = TF + 1
    slots = []
    for a, b in pairs:
        na, nb = sizes[a], sizes[b]
        (sa, ea), (sb, eb) = ranges[a], ranges[b]
        r = na % F
        if r == 0:
            ta = na // F
            a_seg = [(sa, ea)]
            b_seg = [(sb, eb)]
            h_seg = [("dup", sb, HALF)]
            a_t = list(range(ta))
            b_t = list(range(ta, TF)) + [TF]
        elif r <= HALF:
            ta = na // F
            a_seg = [(sa, sa + ta * F)]
            b_seg = [(sb, eb), ("dup", sb, r)]
            h_seg = [(sa + ta * F, ea), ("dup", sa, HALF - r)]
            a_t = list(range(ta)) + [TF]
            b_t = list(range(ta, TF))
        else:
            rb = F - r
            ta1 = na // F + 1
            a_seg = [(sa, ea), ("dup", sa, rb)]
            b_seg = [(sb, eb - rb)]
            h_seg = [(eb - rb, eb), ("dup", sb, HALF - rb)]
            a_t = list(range(ta1))
            b_t = list(range(ta1, TF)) + [TF]
        # core xs = a_seg cols + b_seg cols + h_seg cols (= TF*F + HALF)
        slots.append([(a, a_seg, a_t), (b, b_seg, b_t), (None, h_seg, None)])
    return T, slots


def _core_inputs(x, T, core_slots, weights):
    """Build the per-core input dict (xs + masks); weights are shared."""
    from ml_dtypes import bfloat16

    if not _EXACT[0]:
        xs = np.empty((3, T * F), np.float32)
        mneg = np.full((1, 2 * T), NEG, np.float32)
        m01 = np.zeros((1, 2 * T), np.float32)
        t0 = 0
        for slot, (cid, (s, e), ktiles) in enumerate(core_slots):
            n = e - s
            cols = x[:, s:e] if n > 0 else x[:, :1]
            n = max(n, 1)
            pad = ktiles * F - n
            if pad > 0:
                cols = np.concatenate(
                    [cols, np.repeat(cols[:, :1], pad, axis=1)], axis=1
                )
            xs[:, t0 * F : (t0 + ktiles) * F] = cols
            mneg[0, slot * T + t0 : slot * T + t0 + ktiles] = 0.0
            m01[0, slot * T + t0 : slot * T + t0 + ktiles] = 1.0
            t0 += ktiles
    else:
        parts = []
        mneg = np.full((1, 2 * T), NEG, np.float32)
        m01 = np.zeros((1, 2 * T), np.float32)
        for slot, (cid, segs, tiles) in enumerate(core_slots):
            for seg in segs:
                if seg[0] == "dup":
                    _, at, cnt = seg
                    parts.append(np.repeat(x[:, at : at + 1], cnt, axis=1))
                else:
                    lo, hi = seg
                    parts.append(x[:, lo:hi])
            if tiles is not None:
                for t in tiles:
                    mneg[0, slot * T + t] = 0.0
                    m01[0, slot * T + t] = 1.0
        xs = np.concatenate(parts, axis=1)
        assert xs.shape[1] == (T - 1) * F + HALF, xs.shape
    weights = dict(weights)
    rowcat = np.concatenate([weights.pop("rowpre"), mneg], axis=1)
    return dict(
        xs=xs.astype(bfloat16),
        rowcat=np.ascontiguousarray(rowcat).astype(bfloat16),
        m01=m01.astype(bfloat16),
        **weights,
    )


def _build_nc(T):
    """Build + compile the SPMD Bass program for a per-core tile count T.

    Single-compute structure: pass 1 runs layers 1-2 once per tile, stashing
    f (bf16) in SBUF and per-tile maxes Mt; the combine turns Mt into the
    per-tile L3 bias table Cb; pass 2 reads the stashed f for layers 3-4.
    All matmul operands are bf16 (full-rate streaming + fast weight load);
    PSUM accumulation stays fp32, so only operand quantization is lost.
    """
    import concourse.mybir as mybir
    import concourse.tile as tile
    from concourse import bacc

    f32 = mybir.dt.float32
    f32r = mybir.dt.float32r
    bf16 = mybir.dt.bfloat16
    AF = mybir.ActivationFunctionType
    AX = mybir.AxisListType.X

    nc = bacc.Bacc("TRN2", target_bir_lowering=False, debug=False, num_devices=NCORES)

    last_w = HALF if _EXACT[0] else F
    XC = (T - 1) * F + last_w

    def w_of(t):
        return last_w if t == T - 1 else F

    xs_d = nc.dram_tensor("xs", [3, XC], bf16, kind="ExternalInput")
    w1t_d = nc.dram_tensor("w1t", [3, 128], bf16, kind="ExternalInput")
    w2t_d = nc.dram_tensor("w2t", [128, 256], bf16, kind="ExternalInput")
    w3gt_d = nc.dram_tensor("w3gt", [128, 2, 512], bf16, kind="ExternalInput")
    w32t_d = nc.dram_tensor("w32t", [128, 512], bf16, kind="ExternalInput")
    w4t_d = nc.dram_tensor("w4t", [128, 4, 1024], bf16, kind="ExternalInput")
    bc_d = nc.dram_tensor("bcat", [128, 3], f32, kind="ExternalInput")
    rc_d = nc.dram_tensor("rowcat", [1, 640 + 2 * T], bf16, kind="ExternalInput")
    m01_d = nc.dram_tensor("m01", [1, 2 * T], bf16, kind="ExternalInput")
    vt_d = nc.dram_tensor("vt", [128, T, 8], bf16, kind="ExternalOutput")

    # first x chunk tiny so tile 0 can start early; rest split for overlap
    bounds = [0, F, 2 * F, 4 * F]
    step = max(F, (XC - 4 * F + 3) // 4)
    while bounds[-1] < XC:
        bounds.append(min(XC, bounds[-1] + step))

    with tile.TileContext(nc) as tc:
        with (
            tc.tile_pool(name="const", bufs=1) as cp,
            tc.tile_pool(name="work", bufs=4) as wp,
            tc.tile_pool(name="h3p", bufs=3) as h3p,
        ):
            xs = cp.tile([3, XC], bf16)
            w1t = cp.tile([3, 128], bf16)
            w2t = cp.tile([128, 256], bf16)
            w3gt = cp.tile([128, 2, 512], bf16)
            w32t = cp.tile([128, 512], bf16)
            w4t = cp.tile([128, 4, 1024], bf16)
            bc = cp.tile([128, 3], f32)
            rc = cp.tile([1, 640 + 2 * T], bf16)
            m01t = cp.tile([1, 2 * T], bf16)
            b1 = bc[:, 0:1]
            b2 = bc[:, 1:3]
            b3r = rc[:, 0:512]
            ones = rc[:, 512:640]
            mneg = rc[:, 640 : 640 + 2 * T]
            m01 = m01t[:]
            hsb = cp.tile([128, T, F], bf16)  # stashed layer-1 out h
            Mt = cp.tile([128, T, 2], f32)
            gk = cp.tile([128, 2, 2], bf16)
            cT0 = cp.tile([1, 512], bf16)
            cT1 = cp.tile([1, 512], bf16)
            Cb = cp.tile([128, 4, T], f32)
            V = cp.tile([128, T, 8], bf16)

            # x chunks on gpsimd (SWDGE) run parallel to weights on sync
            # (HWDGE); pass-1-critical tensors first on each queue.
            nc.sync.dma_start(xs[:, 0 : bounds[1]], xs_d.ap()[:, 0 : bounds[1]])
            for a, b_ in zip(bounds[1:], bounds[2:]):
                nc.gpsimd.dma_start(xs[:, a:b_], xs_d.ap()[:, a:b_])
            nc.scalar.dma_start(w1t[:], w1t_d.ap())
            nc.scalar.dma_start(bc[:], bc_d.ap())
            nc.sync.dma_start(w2t[:], w2t_d.ap())
            nc.sync.dma_start(rc[:], rc_d.ap())
            nc.sync.dma_start(m01t[:], m01_d.ap())
            for t_sb, t_dr in ((w3gt, w3gt_d), (w32t, w32t_d), (w4t, w4t_d)):
                nc.sync.dma_start(t_sb[:], t_dr.ap())

            # pre-warm ACT function tables while the DMAs stream in
            warm = wp.tile([128, 1], f32, tag="gtmp")
            nc.gpsimd.memset(warm[:], 0.0)
            nc.scalar.activation(warm[:], warm[:], AF.Relu, bias=warm[:])
            nc.scalar.activation(warm[:], warm[:], AF.Identity, bias=warm[:])

            # ---- single psum scope for the whole kernel ------------------
            # tag "pa": [128, 2, F] 2-bank tiles, bufs=3 (pass-1 L2 pairs and
            # pass-2 L4 pairs); tag "pb": [128, F] 1-bank tiles, bufs=2
            # (L1, combine smalls, pass-2 L3). 6 + 2 = 8 banks.
            LOOK = 2
            with tc.tile_pool(name="psum", bufs=1, space="PSUM") as pp:

                def l1(u):
                    wu = w_of(u)
                    ph = pp.tile([128, wu], f32, tag="pb", bufs=2,
                                 name=f"p1h_{u}")
                    nc.tensor.matmul(
                        ph[:], w1t[:], xs[:, u * F : u * F + wu],
                        start=True, stop=True,
                    )
                    nc.scalar.activation(
                        hsb[:, u, 0:wu], ph[:], AF.Relu, bias=b1
                    )

                for u in range(min(LOOK, T)):
                    l1(u)
                for t in range(T):
                    if t + LOOK < T:
                        l1(t + LOOK)
                    wt = w_of(t)
                    pf = pp.tile([128, 2, F], f32, tag="pa", bufs=3,
                                 name=f"pf_{t}")
                    for m in range(2):
                        nc.tensor.matmul(
                            pf[:, m, 0:wt], w2t[:, m * 128 : (m + 1) * 128],
                            hsb[:, t, 0:wt], start=True, stop=True,
                        )
                    nc.vector.reduce_max(Mt[:, t, 0:2], pf[:, :, 0:wt], axis=AX)

                # ---- combine, interleaved with early pass-2 L3 matmuls ----
                # L3 psums live in "pa" pairs (mo01 / mo23) so the early-L3
                # matmuls never borrow the combine's "pb" rotation (which
                # would deadlock the in-order PE queue against Cb).
                def l3mms(t, half):
                    wt = w_of(t)
                    p3 = pp.tile([128, 2, F], f32, tag="pa", bufs=3,
                                 name=f"p3_{t}_{half}")
                    for sub in range(2):
                        mo = 2 * half + sub
                        nc.tensor.matmul(
                            p3[:, sub, 0:wt], w32t[:, mo * 128 : (mo + 1) * 128],
                            hsb[:, t, 0:wt], start=True, stop=True,
                        )
                    return p3

                def l3acts(t, h3, half, p3):
                    wt = w_of(t)
                    for sub in range(2):
                        mo = 2 * half + sub
                        nc.scalar.activation(
                            h3[:, mo, 0:wt], p3[:, sub, 0:wt], AF.Relu,
                            bias=Cb[:, mo, t : t + 1],
                        )

                def l3pair(t, h3, half):
                    l3acts(t, h3, half, l3mms(t, half))

                h3s = {0: h3p.tile([128, 4, F], bf16, tag="h3", name="h3_0"),
                       1: h3p.tile([128, 4, F], bf16, tag="h3", name="h3_1")}

                for m in range(2):
                    pmask = pp.tile([128, 2 * T], f32, tag="pb", bufs=2)
                    nc.tensor.matmul(pmask[:], ones, mneg, start=True, stop=True)
                    cmb = wp.tile([128, 2 * T], f32, tag="cmb")
                    for sl in range(2):
                        nc.vector.tensor_add(
                            cmb[:, sl * T : (sl + 1) * T], Mt[:, :, m],
                            pmask[:, sl * T : (sl + 1) * T],
                        )
                    for sl in range(2):
                        gtmp = wp.tile([128, 1], f32, tag="gtmp")
                        nc.vector.reduce_max(
                            gtmp[:], cmb[:, sl * T : (sl + 1) * T], axis=AX
                        )
                        nc.vector.tensor_add(
                            gk[:, m, sl : sl + 1], gtmp[:], b2[:, m : m + 1]
                        )

                # independent early-L3 matmuls keep the PE busy while the
                # DVE finishes the g chain; their ACTs (which read Cb) are
                # emitted after the Cb writes below
                early = [(0, 0, l3mms(0, 0)), (0, 1, l3mms(0, 1)),
                         (1, 0, l3mms(1, 0))]

                # c rows live on partition 0 so outer-product lhsT is legal;
                # the b3 row folds into the DVE copy instead of a third matmul
                for sl, cTs in ((0, cT0), (1, cT1)):
                    pcT = pp.tile([1, 512], f32, tag="pb", bufs=2)
                    nc.tensor.matmul(
                        pcT[:], gk[:, 0, sl : sl + 1], w3gt[:, 0, :],
                        start=True, stop=False,
                    )
                    nc.tensor.matmul(
                        pcT[:], gk[:, 1, sl : sl + 1], w3gt[:, 1, :],
                        start=False, stop=True,
                    )
                    nc.vector.tensor_add(cTs[:], pcT[:], b3r)

                for mo in range(4):
                    pC = pp.tile([128, T], f32, tag="pb", bufs=2)
                    nc.tensor.matmul(
                        pC[:], cT0[:, mo * 128 : (mo + 1) * 128], m01[:, 0:T],
                        start=True, stop=False,
                    )
                    nc.tensor.matmul(
                        pC[:], cT1[:, mo * 128 : (mo + 1) * 128],
                        m01[:, T : 2 * T], start=False, stop=True,
                    )
                    nc.vector.tensor_copy(Cb[:, mo, :], pC[:])

                for t_, half_, p3_ in early:
                    l3acts(t_, h3s[t_], half_, p3_)
                l3pair(1, h3s[1], 1)

                # ---- pass 2: L4 of tile t with L3 halves of t+1 / t+2
                # interleaved between the L4 pair streams.
                for t in range(T):
                    h3cur = h3s.pop(t)
                    wt = w_of(t)
                    for pair in range(4):
                        p4 = pp.tile([128, 2, F], f32, tag="pa", bufs=3,
                                     name=f"p4_{t}_{pair}")
                        for sub in range(2):
                            mo = 2 * pair + sub
                            for k in range(4):
                                nc.tensor.matmul(
                                    p4[:, sub, 0:wt],
                                    w4t[:, k, mo * 128 : (mo + 1) * 128],
                                    h3cur[:, k, 0:wt], start=(k == 0), stop=(k == 3),
                                )
                        nc.vector.reduce_max(
                            V[:, t, 2 * pair : 2 * pair + 2], p4[:, :, 0:wt],
                            axis=AX,
                        )
                        if pair == 0 and t + 2 < T:
                            h3s[t + 2] = h3p.tile([128, 4, F], bf16, tag="h3",
                                                  name=f"h3_{t + 2}")
                            l3pair(t + 2, h3s[t + 2], 0)
                        elif pair == 2 and t + 2 < T:
                            l3pair(t + 2, h3s[t + 2], 1)
                    if t % 2 == 1 or t == T - 1:
                        a = (t // 2) * 2
                        nc.sync.dma_start(
                            vt_d.ap()[:, a : t + 1, :], V[:, a : t + 1, :]
                        )

    nc.compile()
    return nc


def _prep(x, npts, W1, b1, g1, be1, m1, v1, W2, b2, g2, be2, m2, v2,
          W3, b3, g3, be3, m3, v3, W4, b4, g4, be4, m4, v4):
    """Host-side preprocessing shared by kernel() and the test harness."""
    W1f, b1f = _fold_bn(W1, b1, g1, be1, m1, v1)
    W2f, b2f = _fold_bn(W2, b2, g2, be2, m2, v2)
    W3f_, b3f = _fold_bn(W3, b3, g3, be3, m3, v3)
    W4f, b4f = _fold_bn(W4, b4, g4, be4, m4, v4)

    from ml_dtypes import bfloat16

    weights = dict(
        w1t=np.ascontiguousarray(W1f.T).astype(bfloat16),
        w2t=np.ascontiguousarray(W2f.T).astype(bfloat16),
        w3gt=np.ascontiguousarray(W3f_[:, :256].T.reshape(2, 128, 512).transpose(1, 0, 2)).astype(bfloat16),
        w32t=np.ascontiguousarray((W3f_[:, 256:] @ W2f).T).astype(bfloat16),
        w4t=np.ascontiguousarray(W4f.T.reshape(4, 128, 1024).transpose(1, 0, 2)).astype(bfloat16),
        bcat=np.ascontiguousarray(
            np.concatenate([b1f[:, None], b2f.reshape(2, 128).T], axis=1)
        ),
        # fsb holds raw W2'h (no b2): fold W3f @ b2 into the b3 row
        rowpre=np.concatenate(
            [(b3f + W3f_[:, 256:] @ b2f)[None, :].astype(np.float32),
             np.ones((1, 128), np.float32)], axis=1
        ),
    )

    T, slots = _plan(npts)
    x = np.asarray(x, np.float32)
    in_maps = [_core_inputs(x, T, slots, weights) for slots in slots]
    return T, slots, in_maps, b4f


def _gather(results, T, slots, b4f):
    """Combine per-core per-tile maxes into the [B, 1024] output."""
    out = np.empty((B, 1024), np.float32)
    for c, core_slots in enumerate(slots):
        vt = np.asarray(results[c]["vt"], np.float32)  # [128, T, 8]
        chan = vt.transpose(2, 0, 1).reshape(1024, T)
        if not _EXACT[0]:
            t0 = 0
            for cid, _rng, ktiles in core_slots:
                out[cid] = chan[:, t0 : t0 + ktiles].max(axis=1) + b4f
                t0 += ktiles
        else:
            for cid, _segs, tiles in core_slots:
                if cid is None:
                    continue
                out[cid] = chan[:, tiles].max(axis=1) + b4f
    return out


def kernel(**inputs):
    from concourse.bass_utils import run_bass_kernel_spmd

    # force host numpy: jax arrays would route host math through the (axon)
    # device backend
    inputs = {k: np.asarray(v) for k, v in inputs.items()}
    T, slots, in_maps, b4f = _prep(**inputs)
    nc = _build_nc(T)
    res = run_bass_kernel_spmd(nc, in_maps, core_ids=list(range(NCORES)))
    return _gather(res.results, T, slots, b4f)

